# revision 12
# baseline (speedup 1.0000x reference)
"""MultiHeadLinearAttention Trainium2 Bass kernel — 8-core SPMD.

Problem (per reference):
  q = elu(LN(Xq @ Wq.T + bq)) + 1 ; k = elu(LN(Xk @ Wk.T + bk)) + 1
  v = Xv @ Wv.T + bv
  kv = sum_n k[n] (x) v[n]   (per head, [D,D]);  ksum = sum_n k[n]
  out = ((q @ kv) / (q . ksum + 1e-8)) @ Wo.T + bo

Sharding: core c -> batch b = c//2, token half h = c%2 (2048 q AND k/v
tokens each). Per-pair (cores 2b, 2b+1) AllReduce of kv/ksum partials
(~266 KB) completes the sum over all 4096 k/v tokens of the batch.

Layouts on chip (per core):
  k,v: [tok x feat] (LN over free dim; kv contraction over token partitions)
  q:   [feat x tok] (q^T feeds num = kv_bd^T @ q^T and out-proj lhsT)
LayerNorm mean is folded into the weights on host (W~ = W^T(I-J/E),
b~ = b - mean(b)); gq/gk==1, betaq/betak==0 (asserted) so
LN(y) = (y - mu(y)) * rsqrt(var + eps) = u * exp(-0.5*ln(mean(u^2)+eps)).
elu(z)+1 = exp(min(z,0)) + relu(z).

All matmuls run as float32r (FP32 bits read at FP22 precision, full PE
rate at moving-dim >= 256).
"""

import os

import numpy as np

B, NSEQ, E, H, D = 4, 4096, 1024, 16, 64
NCORES = 8
T = NSEQ // 2          # tokens per core
TT = T // 128          # token tiles (16)
EI = E // 128          # feature tiles (8)
LN_EPS = 1e-5

_NC_CACHE = {}


def _build_nc(dbg=False):
    from concourse import bacc
    import concourse.bass as bass
    import concourse.mybir as mybir
    import concourse.tile as tile

    f32 = mybir.dt.float32
    f32r = mybir.dt.float32r
    Alu = mybir.AluOpType
    Act = mybir.ActivationFunctionType
    RG = [[0, 1], [2, 3], [4, 5], [6, 7]]

    def r(ap):
        return ap.bitcast(f32r)

    nc = bacc.Bacc(num_devices=NCORES)

    xqT = nc.dram_tensor("xqT", [E, T], f32r, kind="ExternalInput")
    xkT = nc.dram_tensor("xkT", [E, T], f32r, kind="ExternalInput")
    xvT = nc.dram_tensor("xvT", [E, T], f32r, kind="ExternalInput")
    wqT = nc.dram_tensor("wqT", [E, E], f32r, kind="ExternalInput")
    wkT = nc.dram_tensor("wkT", [E, E], f32r, kind="ExternalInput")
    wvT = nc.dram_tensor("wvT", [E, E], f32r, kind="ExternalInput")
    woT = nc.dram_tensor("woT", [E, E], f32r, kind="ExternalInput")
    bq2d = nc.dram_tensor("bq2d", [128, EI], f32, kind="ExternalInput")
    bkR = nc.dram_tensor("bkR", [1, E], f32r, kind="ExternalInput")
    bvR = nc.dram_tensor("bvR", [1, E], f32r, kind="ExternalInput")
    boR = nc.dram_tensor("boR", [1, E], f32r, kind="ExternalInput")
    onesR = nc.dram_tensor("onesR", [1, 128], f32r, kind="ExternalInput")
    onesC = nc.dram_tensor("onesC", [128, 1], f32r, kind="ExternalInput")
    zerosBD = nc.dram_tensor("zerosBD", [128, E], f32r, kind="ExternalInput")
    out_d = nc.dram_tensor("out", [T, E], f32, kind="ExternalOutput")
    if dbg:
        dbg_ar = nc.dram_tensor("dbg_ar", [128, 520], f32, kind="ExternalOutput")
        dbg_pack = nc.dram_tensor("dbg_pack", [128, 520], f32, kind="ExternalOutput")
        dbg_kf = nc.dram_tensor("dbg_kf", [128, E], f32, kind="ExternalOutput")
        dbg_vf = nc.dram_tensor("dbg_vf", [128, E], f32, kind="ExternalOutput")
        dbg_kvbd = nc.dram_tensor("dbg_kvbd", [128, E], f32, kind="ExternalOutput")
        dbg_ks2 = nc.dram_tensor("dbg_ks2", [128, 16], f32, kind="ExternalOutput")
        dbg_rstd = nc.dram_tensor("dbg_rstd", [1, T], f32, kind="ExternalOutput")
        dbg_den = nc.dram_tensor("dbg_den", [2, T], f32, kind="ExternalOutput")
        dbg_qf = nc.dram_tensor("dbg_qf", [128, T], f32, kind="ExternalOutput")
        dbg_num = nc.dram_tensor("dbg_num", [128, T], f32, kind="ExternalOutput")

    with tile.TileContext(nc) as tc:
        with tc.tile_pool(name="const", bufs=1) as cp, \
             tc.tile_pool(name="dram", bufs=1, space="DRAM") as dp:
            ones_col = cp.tile([128, 1], f32, tag="ones_col")
            nc.vector.memset(ones_col, 1.0)
            onesR_sb = cp.tile([1, 128], f32r, tag="onesR_sb")
            nc.sync.dma_start(out=onesR_sb, in_=onesR[:, :])
            onesC_sb = cp.tile([128, 1], f32r, tag="onesC_sb")
            nc.sync.dma_start(out=onesC_sb, in_=onesC[:, :])
            zrow_sb = cp.tile([1, E], f32r, tag="zrow_sb")
            nc.sync.dma_start(out=zrow_sb, in_=zerosBD[0:1, :])
            eps_sb = cp.tile([128, 1], f32, tag="eps_sb")
            nc.vector.memset(eps_sb, LN_EPS)
            eps8_sb = cp.tile([128, 1], f32, tag="eps8_sb")
            nc.vector.memset(eps8_sb, 1e-8)
            bq_sb = cp.tile([128, EI], f32, tag="bq_sb")
            nc.sync.dma_start(out=bq_sb, in_=bq2d[:, :])
            bk_sb = cp.tile([1, E], f32r, tag="bk_sb")
            nc.sync.dma_start(out=bk_sb, in_=bkR[:, :])
            bv_sb = cp.tile([1, E], f32r, tag="bv_sb")
            nc.sync.dma_start(out=bv_sb, in_=bvR[:, :])
            bo_sb = cp.tile([1, E], f32r, tag="bo_sb")
            nc.sync.dma_start(out=bo_sb, in_=boR[:, :])
            kvbd = cp.tile([128, E], f32r, tag="kvbd")
            ar_sb = cp.tile([128, 520], f32, tag="ar_sb")
            rstd_b = cp.tile([128, T], f32, tag="rstd_b")
            cc_in = dp.tile([128, 520], f32, tag="cc_in")
            cc_out = dp.tile([128, 520], f32, tag="cc_out")
            rstd_d = dp.tile([1, T], f32, tag="rstd_d")

            # ---------------- Phase A: k/v proj + LN + elu + kv/ksum ----
            xkT_v = xkT.rearrange("(i p) n -> p i n", p=128)
            xvT_v = xvT.rearrange("(i p) n -> p i n", p=128)
            with tc.tile_pool(name="pa1", bufs=1) as pa1, \
                 tc.tile_pool(name="pa", bufs=2) as pa, \
                 tc.tile_pool(name="psa", bufs=1, space="PSUM") as psa:
                wk_sb = pa1.tile([128, EI, E], f32r, tag="wk")
                nc.sync.dma_start(out=wk_sb, in_=wkT.rearrange("(i p) j -> p i j", p=128))
                wv_sb = pa1.tile([128, EI, E], f32r, tag="wv")
                nc.sync.dma_start(out=wv_sb, in_=wvT.rearrange("(i p) j -> p i j", p=128))

                kv_ps = [psa.tile([128, 512], f32, tag=f"kv{q}", name=f"kv{q}") for q in range(4)]
                ksum_ps = psa.tile([128, 8], f32, tag="ksum")
                for q in range(4):
                    nc.tensor.matmul(kv_ps[q], onesR_sb[:, :], zrow_sb[:, 0:512],
                                     start=True, stop=False,
                                     skip_group_check=True)
                nc.tensor.matmul(ksum_ps, onesR_sb[:, :], zrow_sb[:, 0:8],
                                 start=True, stop=False, skip_group_check=True)

                for t in range(TT):
                    ts = slice(128 * t, 128 * t + 128)
                    xk = pa.tile([128, EI, 128], f32r, tag="xk")
                    nc.sync.dma_start(out=xk, in_=xkT_v[:, :, ts])
                    xv = pa.tile([128, EI, 128], f32r, tag="xv")
                    nc.sync.dma_start(out=xv, in_=xvT_v[:, :, ts])

                    # k projection into [tok x feat] psum (+ rank-1 bias)
                    k_ps = psa.tile([128, E], f32, tag="kps")
                    for i in range(EI):
                        for jh in range(2):
                            js = slice(512 * jh, 512 * jh + 512)
                            nc.tensor.matmul(
                                k_ps[:, js], r(xk[:, i, :]), r(wk_sb[:, i, js]),
                                start=(i == 0), stop=False)
                    for jh in range(2):
                        js = slice(512 * jh, 512 * jh + 512)
                        nc.tensor.matmul(
                            k_ps[:, js], onesR_sb[:, :], bk_sb[:, js],
                            start=False, stop=True)

                    # LN stats + rstd
                    ss = pa.tile([128, 1], f32, tag="ss")
                    scrap = pa.tile([128, E], f32, tag="scrap")
                    nc.scalar.activation(out=scrap, in_=k_ps, func=Act.Square,
                                         accum_out=ss)
                    rstd = pa.tile([128, 1], f32, tag="rstd")
                    nc.scalar.activation(out=rstd, in_=ss, func=Act.Ln,
                                         scale=1.0 / E, bias=eps_sb)
                    nc.scalar.activation(out=rstd, in_=rstd, func=Act.Exp,
                                         scale=-0.5)
                    # elu(u*rstd)+1 = exp(min(z,0)) + relu(z)
                    km = pa.tile([128, E], f32, tag="km")
                    nc.vector.tensor_scalar(out=km, in0=k_ps, scalar1=rstd,
                                            scalar2=0.0, op0=Alu.mult, op1=Alu.min)
                    kf = pa.tile([128, E], f32r, tag="kf")
                    nc.scalar.activation(out=kf, in_=k_ps, func=Act.Relu,
                                         scale=rstd)
                    nc.scalar.activation(out=km, in_=km, func=Act.Exp)
                    nc.vector.tensor_tensor(out=kf, in0=kf.bitcast(f32), in1=km, op=Alu.add)

                    # v projection (j-halves sequential through 1-bank psum)
                    vf = pa.tile([128, E], f32r, tag="vf")
                    for jh in range(2):
                        js = slice(512 * jh, 512 * jh + 512)
                        v_ps = psa.tile([128, 512], f32, tag="vps")
                        for i in range(EI):
                            nc.tensor.matmul(
                                v_ps, r(xv[:, i, :]), r(wv_sb[:, i, js]),
                                start=(i == 0), stop=False)
                        nc.tensor.matmul(v_ps, onesR_sb[:, :], bv_sb[:, js],
                                         start=False, stop=True)
                        nc.scalar.activation(out=vf[:, js], in_=v_ps,
                                             func=Act.Copy)

                    if dbg and t == TT - 1:
                        nc.sync.dma_start(out=dbg_kf[:, :], in_=kf.bitcast(f32))
                        nc.sync.dma_start(out=dbg_vf[:, :], in_=vf.bitcast(f32))
                    # kv_sum + ksum accumulation over token tiles
                    for q4 in range(4):
                        vq = r(vf[:, 256 * q4:256 * q4 + 256])
                        for half in range(2):
                            pr = 2 * q4 + half
                            kp = r(kf[:, 128 * pr:128 * pr + 128])
                            nc.tensor.matmul(
                                kv_ps[q4][:, 256 * half:256 * half + 256],
                                kp, vq, start=False, stop=(t == TT - 1),
                                skip_group_check=True)
                            nc.tensor.matmul(
                                ksum_ps[:, pr:pr + 1], kp.bitcast(f32),
                                ones_col[:, :],
                                start=False, stop=(t == TT - 1),
                                skip_group_check=True)

                # pack useful kv blocks + ksum, then pairwise AllReduce
                pack = pa1.tile([128, 520], f32, tag="pack")
                for p in range(8):
                    q4, odd = divmod(p, 2)
                    c = 64 * p
                    if odd == 0:
                        nc.vector.tensor_copy(out=pack[0:64, c:c + 64],
                                              in_=kv_ps[q4][0:64, 0:64])
                        nc.vector.tensor_copy(out=pack[64:128, c:c + 64],
                                              in_=kv_ps[q4][64:128, 64:128])
                    else:
                        nc.vector.tensor_copy(out=pack[0:64, c:c + 64],
                                              in_=kv_ps[q4][0:64, 384:448])
                        nc.vector.tensor_copy(out=pack[64:128, c:c + 64],
                                              in_=kv_ps[q4][64:128, 448:512])
                nc.vector.tensor_copy(out=pack[:, 512:520], in_=ksum_ps[:, :])
                nc.sync.dma_start(out=cc_in, in_=pack)
                if dbg:
                    nc.sync.dma_start(out=dbg_pack[:, :], in_=pack)

            nc.gpsimd.collective_compute(
                "AllReduce", Alu.add, replica_groups=RG,
                ins=[cc_in[:, :]], outs=[cc_out[:, :]])
            nc.sync.dma_start(out=ar_sb, in_=cc_out[:, :])

            # block-diagonal kv for paired num matmuls
            nc.sync.dma_start(out=kvbd, in_=zerosBD[:, :])
            ev_dst = kvbd[0:64, :].rearrange("p (a two c) -> p a two c",
                                             two=2, c=64)[:, :, 0, :]
            nc.vector.tensor_copy(
                out=ev_dst,
                in_=ar_sb[0:64, 0:512].rearrange("p (a c) -> p a c", c=64))
            od_dst = kvbd[64:128, :].rearrange("p (a two c) -> p a two c",
                                               two=2, c=64)[:, :, 1, :]
            nc.vector.tensor_copy(
                out=od_dst,
                in_=ar_sb[64:128, 0:512].rearrange("p (a c) -> p a c", c=64))
            ksum2 = cp.tile([128, 16], f32r, tag="ksum2")
            nc.sync.dma_start(out=ksum2, in_=zerosBD[:, 0:16])
            for jj in range(EI):
                nc.vector.tensor_copy(out=ksum2[0:64, 2 * jj:2 * jj + 1],
                                      in_=ar_sb[0:64, 512 + jj:513 + jj])
                nc.vector.tensor_copy(out=ksum2[64:128, 2 * jj + 1:2 * jj + 2],
                                      in_=ar_sb[64:128, 512 + jj:513 + jj])

            if dbg:
                nc.sync.dma_start(out=dbg_ar[:, :], in_=ar_sb)
                nc.sync.dma_start(out=dbg_kvbd[:, :], in_=kvbd.bitcast(f32))
                nc.sync.dma_start(out=dbg_ks2[:, :], in_=ksum2.bitcast(f32))

            # ---------------- Phase B: q proj + LN + elu + num/den ------
            with tc.tile_pool(name="pu", bufs=1) as pu:
                u_t = [pu.tile([128, T], f32r, tag=f"u{j}", name=f"u{j}") for j in range(EI)]

                with tc.tile_pool(name="pb1", bufs=1) as pb1, \
                     tc.tile_pool(name="pbw", bufs=4) as pbw, \
                     tc.tile_pool(name="pbs", bufs=2) as pbs, \
                     tc.tile_pool(name="psb1", bufs=1, space="PSUM") as psb1:
                    xq = pb1.tile([128, EI, T], f32r, tag="xq")
                    nc.sync.dma_start(
                        out=xq, in_=xqT.rearrange("(i p) n -> p i n", p=128))
                    ssq_ps = psb1.tile([1, T], f32, tag="ssq")
                    rstd_row = pbs.tile([1, T], f32, tag="rstd_row", bufs=1)
                    for j in range(EI):
                        q_ps = psb1.tile([128, T], f32, tag="qps")
                        for i in range(EI):
                            wq_t = pbw.tile([128, 128], f32r, tag="wqt")
                            nc.sync.dma_start(
                                out=wq_t,
                                in_=wqT[128 * i:128 * i + 128,
                                        128 * j:128 * j + 128])
                            for s in range(4):
                                sl = slice(512 * s, 512 * s + 512)
                                nc.tensor.matmul(
                                    q_ps[:, sl], r(wq_t), r(xq[:, i, sl]),
                                    start=(i == 0), stop=(i == EI - 1))
                        usq = pbs.tile([128, T], f32r, tag="usq")
                        nc.scalar.activation(out=usq, in_=q_ps, func=Act.Square,
                                             bias=bq_sb[:, j:j + 1])
                        nc.vector.tensor_scalar_add(out=u_t[j], in0=q_ps,
                                                    scalar1=bq_sb[:, j:j + 1])
                        for s in range(4):
                            sl = slice(512 * s, 512 * s + 512)
                            nc.tensor.matmul(
                                ssq_ps[:, sl], onesC_sb[:, :], usq[:, sl],
                                start=(j == 0), stop=(j == EI - 1),
                                skip_group_check=True)
                    nc.scalar.activation(out=rstd_row, in_=ssq_ps, func=Act.Ln,
                                         scale=1.0 / E, bias=eps_sb[0:1, :])
                    nc.scalar.activation(out=rstd_row, in_=rstd_row,
                                         func=Act.Exp, scale=-0.5)
                    nc.sync.dma_start(out=rstd_d, in_=rstd_row)
                    if dbg:
                        nc.sync.dma_start(out=dbg_rstd[:, :], in_=rstd_row)
                    nc.sync.dma_start(out=rstd_b,
                                      in_=rstd_d.to_broadcast([128, T]))

                numT_t = u_t
                if True:
                    with tc.tile_pool(name="pb2", bufs=2) as pb2, \
                         tc.tile_pool(name="psb2", bufs=1, space="PSUM") as psb2:
                        for j in range(EI):
                            qf = u_t[j]
                            nc.vector.tensor_tensor(out=qf, in0=qf.bitcast(f32),
                                                    in1=rstd_b, op=Alu.mult)
                            m = pb2.tile([128, T], f32, tag="m")
                            nc.vector.tensor_scalar_min(out=m, in0=qf.bitcast(f32),
                                                        scalar1=0.0)
                            nc.scalar.activation(out=m, in_=m, func=Act.Exp)
                            nc.scalar.activation(out=qf, in_=qf.bitcast(f32), func=Act.Relu)
                            nc.vector.tensor_tensor(out=qf, in0=qf.bitcast(f32),
                                                    in1=m, op=Alu.add)
                            # per-head denominator for pair j (den[t,h] differs per head)
                            den_sb = pb2.tile([2, T], f32, tag="den_sb")
                            for s in range(4):
                                sl = slice(512 * s, 512 * s + 512)
                                den_ps = psb2.tile([2, 512], f32, tag="dps",
                                                   bufs=2)
                                nc.tensor.matmul(den_ps,
                                                 r(ksum2[:, 2 * j:2 * j + 2]),
                                                 r(qf[:, sl]),
                                                 start=True, stop=True)
                                nc.scalar.activation(out=den_sb[:, sl],
                                                     in_=den_ps,
                                                     func=Act.Identity,
                                                     bias=eps8_sb[0:2, :])
                            rden = pb2.tile([2, T], f32, tag="rden")
                            nc.vector.reciprocal(out=rden, in_=den_sb)
                            rden_d = dp.tile([2, T], f32, tag="rden_d", bufs=2,
                                             name="rden_d")
                            nc.sync.dma_start(out=rden_d, in_=rden)
                            rden_b = pb2.tile([128, T], f32, tag="rden_b")
                            nc.sync.dma_start(
                                out=rden_b,
                                in_=bass.AP(tensor=rden_d.tensor,
                                            offset=rden_d.offset,
                                            ap=[[T, 2], [0, 64], [1, T]]))
                            nc.vector.tensor_tensor(out=qf, in0=qf.bitcast(f32),
                                                    in1=rden_b, op=Alu.mult)
                            if dbg and j == 0:
                                nc.sync.dma_start(out=dbg_den[:, :], in_=den_sb)
                                nc.sync.dma_start(out=dbg_qf[:, :], in_=qf.bitcast(f32))
                            kv_j = r(kvbd[:, 128 * j:128 * j + 128])
                            for s in range(4):
                                sl = slice(512 * s, 512 * s + 512)
                                num_ps = psb2.tile([128, 512], f32,
                                                   tag="nps", bufs=3)
                                nc.tensor.matmul(num_ps, kv_j, r(qf[:, sl]),
                                                 start=True, stop=True)
                                nc.scalar.activation(out=numT_t[j][:, sl],
                                                     in_=num_ps, func=Act.Copy)
                            if dbg and j == 0:
                                nc.sync.dma_start(out=dbg_num[:, :],
                                                  in_=numT_t[0].bitcast(f32))

                    # ---------------- Phase C: out proj -----------------
                    with tc.tile_pool(name="pc1", bufs=1) as pc1, \
                         tc.tile_pool(name="pc", bufs=2) as pcl, \
                         tc.tile_pool(name="psc", bufs=2, space="PSUM") as psc:
                        wo_sb = pc1.tile([128, EI, E], f32r, tag="wo")
                        nc.sync.dma_start(
                            out=wo_sb,
                            in_=woT.rearrange("(e p) j -> p e j", p=128))
                        for tt in range(TT):
                            tsl = slice(128 * tt, 128 * tt + 128)
                            o_ps = psc.tile([128, E], f32, tag="ops")
                            for e in range(EI):
                                lh = r(numT_t[e][:, tsl])
                                for jh in range(2):
                                    js = slice(512 * jh, 512 * jh + 512)
                                    nc.tensor.matmul(
                                        o_ps[:, js], lh, r(wo_sb[:, e, js]),
                                        start=(e == 0), stop=False)
                            for jh in range(2):
                                js = slice(512 * jh, 512 * jh + 512)
                                nc.tensor.matmul(
                                    o_ps[:, js], onesR_sb[:, :],
                                    bo_sb[:, js], start=False, stop=True)
                            o_sb = pcl.tile([128, E], f32, tag="osb")
                            nc.vector.tensor_copy(out=o_sb, in_=o_ps)
                            nc.sync.dma_start(out=out_d[tsl, :], in_=o_sb)

    nc.finalize()
    return nc


def _prep_inputs(inputs):
    """Host-side fold + per-core shard maps."""
    f = np.float32
    Wq, bq = inputs["Wq"], inputs["bq"]
    Wk, bk = inputs["Wk"], inputs["bk"]
    Wv, bv = inputs["Wv"], inputs["bv"]
    Wo, bo = inputs["Wo"], inputs["bo"]
    for name in ("gq", "gk"):
        assert np.allclose(np.asarray(inputs[name]), 1.0), f"{name} != 1 unsupported"
    for name in ("betaq", "betak"):
        assert np.allclose(np.asarray(inputs[name]), 0.0), f"{name} != 0 unsupported"

    wqT = np.ascontiguousarray(np.asarray(Wq, f).T)
    wqT = wqT - wqT.mean(axis=1, keepdims=True)
    bqf = np.asarray(bq, f) - np.asarray(bq, f).mean()
    wkT = np.ascontiguousarray(np.asarray(Wk, f).T)
    wkT = wkT - wkT.mean(axis=1, keepdims=True)
    bkf = np.asarray(bk, f) - np.asarray(bk, f).mean()
    wvT = np.ascontiguousarray(np.asarray(Wv, f).T)
    woT = np.ascontiguousarray(np.asarray(Wo, f).T)

    shared = {
        "wqT": np.ascontiguousarray(wqT, f),
        "wkT": np.ascontiguousarray(wkT, f),
        "wvT": wvT,
        "woT": woT,
        "bq2d": np.ascontiguousarray(bqf.reshape(EI, 128).T, f),
        "bkR": np.ascontiguousarray(bkf.reshape(1, E), f),
        "bvR": np.ascontiguousarray(np.asarray(bv, f).reshape(1, E)),
        "boR": np.ascontiguousarray(np.asarray(bo, f).reshape(1, E)),
        "onesR": np.ones((1, 128), f),
        "onesC": np.ones((128, 1), f),
        "zerosBD": np.zeros((128, E), f),
    }
    qe = np.asarray(inputs["query_embed"], f)
    ke = np.asarray(inputs["key_embed"], f)
    ve = np.asarray(inputs["value"], f)
    in_maps = []
    for c in range(NCORES):
        b, hh = divmod(c, 2)
        sl = slice(hh * T, (hh + 1) * T)
        m = dict(shared)
        m["xqT"] = np.ascontiguousarray(qe[b, sl, :].T)
        m["xkT"] = np.ascontiguousarray(ke[b, sl, :].T)
        m["xvT"] = np.ascontiguousarray(ve[b, sl, :].T)
        in_maps.append(m)
    return in_maps


def _run(inputs, trace=False):
    from concourse.bass_utils import run_bass_kernel_spmd

    dbg = bool(int(os.environ.get("KERNEL_DBG", "0")))
    key = "nc_dbg" if dbg else "nc"
    if key not in _NC_CACHE:
        _NC_CACHE[key] = _build_nc(dbg=dbg)
    nc = _NC_CACHE[key]
    in_maps = _prep_inputs(inputs)
    res = run_bass_kernel_spmd(nc, in_maps, core_ids=list(range(NCORES)),
                               trace=trace)
    out = np.empty((B, NSEQ, E), np.float32)
    for c in range(NCORES):
        b, hh = divmod(c, 2)
        out[b, hh * T:(hh + 1) * T, :] = res.results[c]["out"]
    return out, res


def kernel(**inputs):
    out, _ = _run(inputs, trace=False)
    return out


def kernel_traced(**inputs):
    """Like kernel() but also returns (exec_time_ns, trace_path)."""
    import sys, types
    try:
        import antenv
        if "antenv.axon_hooks" not in sys.modules:
            mod = types.ModuleType("antenv.axon_hooks")
            _h = [None]
            mod.set_axon_ntff_profile_hook = lambda h: _h.__setitem__(0, h)
            mod.get_axon_ntff_profile_hook = lambda: _h[0]
            sys.modules["antenv.axon_hooks"] = mod
            antenv.axon_hooks = mod
            from trn_agent_boot.trn_boot import _ntff_profile_via_ctypes
            mod.set_axon_ntff_profile_hook(
                _ntff_profile_via_ctypes("/opt/axon/libaxon_pjrt.so"))
    except Exception as e:  # profiling is best-effort
        print(f"NTFF hook setup failed: {e}")
    out, res = _run(inputs, trace=True)
    tp = res.instructions_and_trace[1] if res.instructions_and_trace else None
    return out, res.exec_time_ns, tp
